# revision 10
# baseline (speedup 1.0000x reference)
"""Self-contained TRN2 Bass kernel for nn_MultiHeadAttentionLayer
(GNN multi-head attention message passing), 8 NeuronCores.

kernel(**inputs) takes the FULL unsharded inputs (h, Wq, bq, Wk, bk, Wv,
bv, src, dst) as numpy arrays and returns the FULL [N, H, D] float32
output. Sharding: edges are partitioned by dst range across the 8 cores
(no collectives needed); each core projects K/V for all nodes, gathers
K|V rows per edge with dma_gather, computes scores/softmax weights on
DVE/ACT, and segment-sums into its private dst slice via one-hot
matmuls on the TensorEngine.

v2: fp8 one-hot matrices, -1-trimmed sorted gather indices, biases
restructured out of the hot loops (V-bias via z*bv at finalize, K-bias
via t-columns appended to the Q table, Q-bias via replicated-const
add), broadcast-multiply instead of broadcast-exp, per-block Q-gather
PSUM->SBUF casts on Scalar so the score multiply runs packed-bf16 on
DVE.
"""

from dataclasses import dataclass, field

import numpy as np
import ml_dtypes

import concourse.bass as bass
import concourse.tile as tile
from concourse import bacc, mybir
from concourse.bass import ts
from concourse.bass_utils import run_bass_kernel_spmd

BF16 = ml_dtypes.bfloat16
FP8 = ml_dtypes.float8_e4m3fn
F32 = np.float32
AF = mybir.ActivationFunctionType
ALU = mybir.AluOpType

CH_CAP = 18          # max blocks per gather/compute chunk


@dataclass
class Cfg:
    N: int
    IN: int
    H: int
    D: int
    n_cores: int = 8
    NPC: int = 0
    NT: int = 0
    NSB: int = 0
    CHL: list = field(default_factory=list)   # lo-group blocks per sb
    CHH: list = field(default_factory=list)   # hi-group blocks per sb
    MAXC: list = field(default_factory=list)  # [pos][g] max real count over cores
    ASSIGN: list = field(default_factory=list)  # [core][pos] -> global sb

    @property
    def C(self):
        return self.H * self.D

    @property
    def KA(self):
        return self.IN // 128

    @property
    def NT2(self):
        return self.NT // 2

    @property
    def CZ(self):
        return self.C + self.H

    @property
    def SBLK(self):
        return [l + h for l, h in zip(self.CHL, self.CHH)]


def make_cfg(N, IN, H, D, src, dst, n_cores=8):
    cfg = Cfg(N=N, IN=IN, H=H, D=D, n_cores=n_cores)
    cfg.NPC = -(-N // (n_cores * 128)) * 128
    cfg.NT = cfg.NPC * n_cores
    cfg.NSB = cfg.NPC // 128
    src = np.asarray(src)
    dst = np.asarray(dst)
    gsb = dst // 128                       # global super-block of each edge
    nsb_tot = cfg.NSB * n_cores
    is_hi = (src >= cfg.NT2).astype(np.int64)
    counts = np.zeros((nsb_tot, 2), dtype=np.int64)
    np.add.at(counts, (np.minimum(gsb, nsb_tot - 1), is_hi), 1)
    # balanced assignment: position k across cores gets super-blocks of
    # similar total size -> minimal max-over-cores padding
    order = np.argsort(-(counts[:, 0] + counts[:, 1]), kind="stable")
    cfg.ASSIGN = [[int(order[k * n_cores + i]) for k in range(cfg.NSB)]
                  for i in range(n_cores)]
    cfg.CHL = []
    cfg.CHH = []
    cfg.MAXC = []
    for k in range(cfg.NSB):
        grp = order[k * n_cores:(k + 1) * n_cores]
        ml = max(1, int(counts[grp, 0].max()))
        mh = max(1, int(counts[grp, 1].max()))
        cfg.CHL.append(-(-ml // 128))
        cfg.CHH.append(-(-mh // 128))
        cfg.MAXC.append((ml, mh))
    return cfg


def chunk_plan(cfg):
    """[(j, g, cb, CH, key)] in canonical order; key indexes nval columns."""
    plan = []
    key = 0
    for j in range(cfg.NSB):
        for g, CHG in ((0, cfg.CHL[j]), (1, cfg.CHH[j])):
            cap = CH_CAP if not (g == 1 and j == cfg.NSB - 1) \
                else max(6, -(-CHG // 3))
            b0 = 0
            while b0 < CHG:
                plan.append((j, g, b0, min(cap, CHG - b0), key))
                key += 1
                b0 += cap
    return plan


def _wrap16(idx, epb):
    base = idx.reshape(epb // 16, 16).T.astype(np.int16)
    return np.tile(base, (8, 1))


def prep(cfg: Cfg, h, Wq, bq, Wk, bk, Wv, bv, src, dst):
    N, IN, H, D, C = cfg.N, cfg.IN, cfg.H, cfg.D, cfg.C
    scale = 1.0 / np.sqrt(np.float32(D))

    hT = np.zeros((IN, cfg.NT), dtype=BF16)
    hT[:, :N] = np.asarray(h).T.astype(BF16)
    wkv = np.concatenate([np.asarray(Wk), np.asarray(Wv)], axis=1).astype(BF16)
    wq = (np.asarray(Wq) * scale).astype(BF16)

    # replicated-constant rows (128 x C)
    bq_rep = np.tile((np.asarray(bq) * scale)[None, :], (128, 1)).astype(BF16)
    bk_rep = np.tile(np.asarray(bk)[None, :], (128, 1)).astype(BF16)
    bv_rep = np.tile(np.asarray(bv)[None, :], (128, 1)).astype(F32)

    src = np.asarray(src).astype(np.int64)
    dst = np.asarray(dst).astype(np.int64)

    sum_blk = sum(cfg.SBLK)
    sum_epb = sum_blk * 128
    marange = np.arange(128, dtype=np.int64)

    gsb_of = dst // 128
    in_maps = []
    for i in range(cfg.n_cores):
        srcidx = np.zeros(sum_epb, dtype=np.int64)
        ld = np.full((sum_blk, 128), 255, dtype=np.int64)
        off_e = 0
        off_b = 0
        for j in range(cfg.NSB):
            g_sb = cfg.ASSIGN[i][j]
            insb = gsb_of == g_sb
            es, ed = src[insb], dst[insb] - g_sb * 128
            for g, chg in ((0, cfg.CHL[j]), (1, cfg.CHH[j])):
                if g == 0:
                    gsel = es < cfg.NT2
                    gidx = es[gsel]
                else:
                    gsel = es >= cfg.NT2
                    gidx = es[gsel] - cfg.NT2
                edg = ed[gsel]
                # sort by table row for DRAM locality; pads stay trailing
                o = np.argsort(gidx, kind="stable")
                gidx = gidx[o]
                edg = edg[o]
                cnt = gidx.shape[0]
                epb = chg * 128
                assert cnt <= epb, (i, j, g, cnt, epb)
                srcidx[off_e:off_e + cnt] = gidx
                ldj = np.full(epb, 255, dtype=np.int64)
                ldj[:cnt] = edg
                ld[off_b:off_b + chg, :] = ldj.reshape(chg, 128)
                off_e += epb
                off_b += chg

        srcw_parts = []
        off = 0
        for j in range(cfg.NSB):
            for chg in (cfg.CHL[j], cfg.CHH[j]):
                epb = chg * 128
                srcw_parts.append(_wrap16(srcidx[off:off + epb], epb))
                off += epb
        srcw = np.concatenate(srcw_parts, axis=1)

        onehot = (ld[:, :, None] == marange[None, None, :])       # [bb, e, m]
        Sh = np.ascontiguousarray(onehot.transpose(1, 0, 2)).astype(FP8)
        ShT = np.ascontiguousarray(onehot.transpose(2, 0, 1)).astype(FP8)

        cols = np.concatenate(
            [np.arange(cfg.ASSIGN[i][j] * 128, cfg.ASSIGN[i][j] * 128 + 128)
             for j in range(cfg.NSB)])
        in_maps.append({
            "hT": hT,
            "hTq": np.ascontiguousarray(hT[:, cols]),
            "wkv": wkv, "wq": wq,
            "bq_rep": bq_rep, "bk_rep": bk_rep, "bv_rep": bv_rep,
            "srcidx": srcw,
            "Sh": Sh, "ShT": ShT,
        })
    return in_maps


def build(cfg: Cfg):
    N, IN, H, D, C = cfg.N, cfg.IN, cfg.H, cfg.D, cfg.C
    KA = cfg.KA
    C2 = 2 * C
    CZ = cfg.CZ            # C + H
    sum_blk = sum(cfg.SBLK)
    sum_epb = sum_blk * 128
    bf = mybir.dt.bfloat16
    f8 = mybir.dt.float8e4
    f32 = mybir.dt.float32

    nc = bacc.Bacc("TRN2", target_bir_lowering=False, debug=False)
    hT = nc.dram_tensor("hT", [IN, cfg.NT], bf, kind="ExternalInput").ap()
    hTq = nc.dram_tensor("hTq", [IN, cfg.NPC], bf, kind="ExternalInput").ap()
    wkv = nc.dram_tensor("wkv", [IN, C2], bf, kind="ExternalInput").ap()
    wq = nc.dram_tensor("wq", [IN, C], bf, kind="ExternalInput").ap()
    bq_rep = nc.dram_tensor("bq_rep", [128, C], bf, kind="ExternalInput").ap()
    bk_rep = nc.dram_tensor("bk_rep", [128, C], bf, kind="ExternalInput").ap()
    bv_rep = nc.dram_tensor("bv_rep", [128, C], f32, kind="ExternalInput").ap()
    srcidx = nc.dram_tensor("srcidx", [128, sum_epb // 16], mybir.dt.int16,
                            kind="ExternalInput").ap()
    Sh_d = nc.dram_tensor("Sh", [128, sum_blk, 128], f8, kind="ExternalInput").ap()
    ShT_d = nc.dram_tensor("ShT", [128, sum_blk, 128], f8, kind="ExternalInput").ap()
    out = nc.dram_tensor("out", [cfg.NPC, C], f32, kind="ExternalOutput").ap()

    with tile.TileContext(nc) as tc:
        with (
            tc.tile_pool(name="dram", bufs=1, space="DRAM") as dramp,
            tc.tile_pool(name="const", bufs=1) as constp,
        ):
            kv_lo = dramp.tile([cfg.NT2, C2], bf)
            kv_hi = dramp.tile([cfg.NT2, C2], bf)

            wkvt = constp.tile([128, KA, C2], bf)
            nc.sync.dma_start(wkvt[:], wkv.rearrange("(a p) c -> p a c", p=128))
            wqt = constp.tile([128, KA, C], bf)
            nc.sync.dma_start(wqt[:], wq.rearrange("(a p) c -> p a c", p=128))
            bqr = constp.tile([128, C], bf)
            nc.sync.dma_start(bqr[:], bq_rep[:])
            bkr = constp.tile([128, C], bf)
            nc.sync.dma_start(bkr[:], bk_rep[:])
            bvr = constp.tile([128, C], f32)
            nc.sync.dma_start(bvr[:], bv_rep[:])
            srct = constp.tile([128, sum_epb // 16], mybir.dt.int16)
            nc.sync.dma_start(srct[:], srcidx[:])
            qs = constp.tile([128, cfg.NSB, CZ], bf)

            # ---------------- Phase A ----------------
            import contextlib
            pg_ctx = contextlib.ExitStack()
            pg = pg_ctx.enter_context(tc.tile_pool(name="pb_g", bufs=4))
            with (
                tc.tile_pool(name="pa_h", bufs=1) as pah,
                tc.tile_pool(name="pa_ps", bufs=6, space="PSUM") as paps,
                tc.tile_pool(name="pa_sb", bufs=4) as pasb,
                tc.tile_pool(name="pa_t", bufs=2) as patp,
            ):
                hts = pah.tile([128, KA, cfg.NT], bf)
                hT_r = hT.rearrange("(a p) n -> p a n", p=128)
                NSPL = 8
                SPL = cfg.NT // NSPL
                for sp in range(NSPL):
                    nc.sync.dma_start(hts[:, :, ts(sp, SPL)], hT_r[:, :, ts(sp, SPL)])
                htq = pah.tile([128, KA, cfg.NPC], bf)
                nc.sync.dma_start(htq[:], hTq.rearrange("(a p) n -> p a n", p=128))

                NC2 = cfg.NT2 // 128

                def kv_chunk(cc, tbl):
                    ps = paps.tile([128, C2], f32, tag="psA")
                    for a in range(KA):
                        nc.tensor.matmul(out=ps[:], lhsT=hts[:, a, ts(cc, 128)],
                                         rhs=wkvt[:, a, :], start=(a == 0),
                                         stop=(a == KA - 1))
                    buf = pasb.tile([128, C2], bf, tag="bufA")
                    nc.scalar.copy(buf[:, 0:C], ps[:, 0:C])
                    nc.vector.tensor_copy(buf[:, C:C2], ps[:, C:C2])
                    nc.sync.dma_start(tbl[ts(cc % NC2, 128), :], buf[:])

                for cc in range(NC2):
                    kv_chunk(cc, kv_lo[:])

                for qc in range(cfg.NSB):
                    psq = paps.tile([128, C], f32, tag="psA", name="psq")
                    for a in range(KA):
                        nc.tensor.matmul(out=psq[:], lhsT=htq[:, a, ts(qc, 128)],
                                         rhs=wqt[:, a, :], start=(a == 0),
                                         stop=(a == KA - 1))
                    nc.vector.tensor_tensor(qs[:, qc, 0:C], psq[:], bqr[:],
                                            op=ALU.add)
                    # t-columns: t[m,h] = sum_d bk[h,d] * Qf[m,h,d]
                    tq = patp.tile([128, C], bf, tag="tq")
                    nc.vector.tensor_tensor(tq[:], qs[:, qc, 0:C], bkr[:],
                                            op=ALU.mult)
                    t4 = tq[:].rearrange("p (h d) -> p h d", d=D)
                    cur = t4
                    w = D
                    while w > 2:
                        w //= 2
                        nxt = patp.tile([128, H, w], bf, tag=f"tt{w}")
                        nc.vector.tensor_tensor(
                            nxt[:], cur[:, :, 0:w], cur[:, :, w:2 * w],
                            op=ALU.add)
                        cur = nxt[:]
                    nc.vector.tensor_tensor(
                        qs[:, qc, C:CZ].unsqueeze(2), cur[:, :, 0:1],
                        cur[:, :, 1:2], op=ALU.add)

                for cc in range(NC2, 2 * NC2):
                    kv_chunk(cc, kv_hi[:])

            # ---------------- Phase B ----------------
            grp_off = {}
            off_b = 0
            for j in range(cfg.NSB):
                grp_off[(j, 0)] = off_b
                off_b += cfg.CHL[j]
                grp_off[(j, 1)] = off_b
                off_b += cfg.CHH[j]

            with (
                tc.tile_pool(name="pb_t", bufs=2) as pt,
                tc.tile_pool(name="pb_c", bufs=2) as pc,
                tc.tile_pool(name="pb_w", bufs=2) as pw,
                tc.tile_pool(name="pb_s", bufs=2) as psm,
                tc.tile_pool(name="pb_ps", bufs=4, space="PSUM") as pps,
                tc.tile_pool(name="pb_qps", bufs=4, space="PSUM") as pqps,
            ):
                pswz_of = {}

                plan_all = chunk_plan(cfg)

                def process_group(j, g, tbl, last_of_sb):
                    gb = grp_off[(j, g)]
                    if j not in pswz_of:
                        pswz_of[j] = pps.tile([128, 512], f32, tag="pswz",
                                              name=f"pswz{j}")
                    pswz = pswz_of[j]
                    first_of_sb = (g == 0)
                    chunks = [(cb, CH, key) for (jj, gg, cb, CH, key) in plan_all
                              if jj == j and gg == g]
                    for (ci, (cb, CH, key)) in enumerate(chunks):
                        cbk = gb + cb
                        ce = cbk * 128
                        EPC = CH * 128
                        kvg = pg.tile([128, CH, C2], bf, tag="kvg")
                        nc.gpsimd.dma_gather(
                            kvg[:], tbl, srct[:, ce // 16:(ce + EPC) // 16],
                            EPC, EPC, C2, single_packet=False)
                        sh = pt.tile([128, CH, 128], f8, tag="sh")
                        nc.sync.dma_start(sh[:], Sh_d[:, cbk:cbk + CH, :])
                        sht = pt.tile([128, CH, 128], f8, tag="sht")
                        nc.sync.dma_start(sht[:], ShT_d[:, cbk:cbk + CH, :])

                        # Q-gather (PE) + per-block cast to SBUF (Scalar);
                        # no dependency on the K|V gather
                        qg = pc.tile([128, CH, CZ], bf, tag="qg")
                        for b in range(CH):
                            qps = pqps.tile([128, 512], f32, tag="qps")
                            nc.tensor.matmul(out=qps[:, 0:CZ],
                                             lhsT=sht[:, b, :],
                                             rhs=qs[:, j, :],
                                             start=True, stop=True)
                            nc.scalar.copy(qg[:, b, :], qps[:, 0:CZ])

                        # score: P = Kg * Qg (packed bf16), tree-sum over D
                        P = pc.tile([128, CH, C], bf, tag="P")
                        nc.vector.tensor_tensor(P[:], kvg[:, :, 0:C],
                                                qg[:, :, 0:C], op=ALU.mult)
                        P4 = P[:].rearrange("p b (h d) -> p b h d", d=D)
                        cur = P4
                        w = D
                        while w > 2:
                            w //= 2
                            nxt = pc.tile([128, CH, H, w], bf, tag=f"s{w}")
                            nc.vector.tensor_tensor(
                                nxt[:], cur[:, :, :, 0:w], cur[:, :, :, w:2 * w],
                                op=ALU.add)
                            cur = nxt[:]
                        sc = pc.tile([128, CH, H], f32, tag="sc")
                        nc.vector.tensor_tensor(
                            sc[:].unsqueeze(3), cur[:, :, :, 0:1], cur[:, :, :, 1:2],
                            op=ALU.add)
                        # + t-term (K-bias), clamp to +-5 as in reference
                        sc2 = pc.tile([128, CH, H], f32, tag="sc2")
                        nc.vector.tensor_tensor(sc2[:], sc[:], qg[:, :, C:CZ],
                                                op=ALU.add)
                        scc = pc.tile([128, CH, H], f32, tag="scc")
                        nc.vector.tensor_scalar(scc[:], sc2[:], 5.0, -5.0,
                                                op0=ALU.min, op1=ALU.max)

                        wvz = pw.tile([128, CH, CZ], bf, tag="wvz")
                        nc.scalar.activation(wvz[:, :, C:CZ], scc[:], AF.Exp)
                        nc.vector.tensor_tensor(
                            wvz[:, :, 0:C].rearrange("p b (h d) -> p b h d", d=D),
                            kvg[:, :, C:C2].rearrange("p b (h d) -> p b h d", d=D),
                            wvz[:, :, C:CZ].unsqueeze(3).broadcast_to(
                                [128, CH, H, D]),
                            op=ALU.mult)

                        for b in range(CH):
                            nc.tensor.matmul(
                                out=pswz[:, 0:CZ], lhsT=sh[:, b, :],
                                rhs=wvz[:, b, :],
                                start=(first_of_sb and ci == 0 and b == 0),
                                stop=(last_of_sb and ci == len(chunks) - 1
                                      and b == CH - 1))

                def finalize(j):
                    pswz = pswz_of.pop(j)
                    zm = psm.tile([128, H], f32, tag="zm")
                    nc.vector.tensor_scalar(zm[:], pswz[:, C:CZ], 1e-30, None,
                                            op0=ALU.max)
                    zr = psm.tile([128, H], f32, tag="zr")
                    nc.vector.reciprocal(zr[:], zm[:])
                    # wV' = wV + z*bv  (keeps isolated nodes exactly 0)
                    zbv = psm.tile([128, C], f32, tag="zbv")
                    nc.vector.tensor_tensor(
                        zbv[:].rearrange("p (h d) -> p h d", d=D),
                        bvr[:].rearrange("p (h d) -> p h d", d=D),
                        pswz[:, C:CZ].unsqueeze(2).broadcast_to([128, H, D]),
                        op=ALU.mult)
                    wv2 = psm.tile([128, C], f32, tag="wv2")
                    nc.vector.tensor_tensor(wv2[:], pswz[:, 0:C], zbv[:],
                                            op=ALU.add)
                    of = psm.tile([128, C], f32, tag="of")
                    nc.vector.tensor_tensor(
                        of[:].rearrange("p (h d) -> p h d", d=D),
                        wv2[:].rearrange("p (h d) -> p h d", d=D),
                        zr[:].unsqueeze(2).broadcast_to([128, H, D]),
                        op=ALU.mult)
                    nc.sync.dma_start(out[ts(j, 128), :], of[:])

                NLEAD = min(3, cfg.NSB)
                for j in range(NLEAD):
                    process_group(j, 0, kv_lo[:], last_of_sb=False)
                for k in range(cfg.NSB):
                    process_group(k, 1, kv_hi[:], last_of_sb=True)
                    finalize(k)
                    if k + NLEAD < cfg.NSB:
                        process_group(k + NLEAD, 0, kv_lo[:], last_of_sb=False)
            pg_ctx.close()

    nc.compile()
    return nc


def run(cfg: Cfg, in_maps, trace=False, nc=None):
    if nc is None:
        nc = build(cfg)
    res = run_bass_kernel_spmd(nc, in_maps, core_ids=list(range(cfg.n_cores)),
                               trace=trace)
    full = np.zeros((cfg.NT, cfg.C), dtype=np.float32)
    for i in range(cfg.n_cores):
        o = res.results[i]["out"]
        for j in range(cfg.NSB):
            g_sb = cfg.ASSIGN[i][j]
            full[g_sb * 128:(g_sb + 1) * 128] = o[j * 128:(j + 1) * 128]
    full = full[:cfg.N]
    return full.reshape(cfg.N, cfg.H, cfg.D), res


_PROBLEM_N = 10000
_PROBLEM_IN = 256
_PROBLEM_H = 8
_PROBLEM_D = 32


def kernel(h, Wq, bq, Wk, bk, Wv, bv, src, dst):
    h = np.asarray(h)
    N, IN = h.shape
    C = np.asarray(Wq).shape[1]
    H, D = _PROBLEM_H, _PROBLEM_D
    if C != H * D:
        D = C // H
    src = np.asarray(src)
    dst = np.asarray(dst)
    cfg = make_cfg(N, IN, H, D, src, dst)
    in_maps = prep(cfg, h, Wq, bq, Wk, bk, Wv, bv, src, dst)
    out, _ = run(cfg, in_maps, trace=False)
    return out.astype(np.float32)


# revision 11
# speedup vs baseline: 1.0426x; 1.0426x over previous
"""Self-contained TRN2 Bass kernel for nn_MultiHeadAttentionLayer
(GNN multi-head attention message passing), 8 NeuronCores.

kernel(**inputs) takes the FULL unsharded inputs (h, Wq, bq, Wk, bk, Wv,
bv, src, dst) as numpy arrays and returns the FULL [N, H, D] float32
output. Sharding: edges are partitioned by dst range across the 8 cores
(no collectives needed); each core projects K/V for all nodes into a
DRAM table, fetches K|V rows per edge with dma_gather, computes
scores/softmax weights on DVE/ACT, and segment-sums into its private
dst slice via one-hot matmuls on the TensorEngine.

The SWDGE descriptor generation on GpSimd (~7ns/descriptor) is the
critical path, so edges are PAIRED: two nodes that co-occur in the same
(dst-superblock, table-half) groups are placed in adjacent table rows
(per-core hT column permutation) and fetched with one 2KB descriptor
serving two edge slots. Exact-bitmask bucket matching pairs ~90% of
nodes, cutting descriptors ~40%.
"""

from dataclasses import dataclass, field

import numpy as np
import ml_dtypes

import concourse.bass as bass
import concourse.tile as tile
from concourse import bacc, mybir
from concourse.bass import ts
from concourse.bass_utils import run_bass_kernel_spmd

BF16 = ml_dtypes.bfloat16
FP8 = ml_dtypes.float8_e4m3fn
F32 = np.float32
AF = mybir.ActivationFunctionType
ALU = mybir.AluOpType

CH_CAP = 18          # max blocks per gather/compute chunk (even)


@dataclass
class Cfg:
    N: int
    IN: int
    H: int
    D: int
    n_cores: int = 8
    NPC: int = 0
    NT: int = 0
    NSB: int = 0
    ASSIGN: list = field(default_factory=list)  # [core][pos] -> global sb
    # static gather layout (shared across cores):
    # regions: list of dicts {h, cls, jsub: [(j, ndesc)], chunks: [...]}
    REGIONS: list = field(default_factory=list)
    SUMBLK: int = 0
    SUMDESC: int = 0

    @property
    def C(self):
        return self.H * self.D

    @property
    def KA(self):
        return self.IN // 128

    @property
    def NT2(self):
        return self.NT // 2

    @property
    def CZ(self):
        return self.C + self.H


def _half_masks_counts(cfg, src, dst, core, half):
    """per-(j) node uses for this core/half: returns list of (nodes, dstoffs)
    per j, and the bitmask per node."""
    NT2 = cfg.NT2
    gsb = dst // 128
    masks = np.zeros(NT2, dtype=np.int64)
    peruse = []
    for j in range(cfg.NSB):
        g_sb = cfg.ASSIGN[core][j]
        sel = gsb == g_sb
        es = src[sel]
        ed = dst[sel] - g_sb * 128
        hsel = (es >= half * NT2) & (es < (half + 1) * NT2)
        es = es[hsel] - half * NT2
        ed = ed[hsel]
        uniq = np.unique(es)
        masks[uniq] |= (1 << j)
        peruse.append((es, ed))
    return masks, peruse


def _match_pairs(masks):
    """exact-bitmask greedy pairing; returns partner[] (-1 unpaired)."""
    NT2 = masks.shape[0]
    partner = np.full(NT2, -1, dtype=np.int64)
    used = np.nonzero(masks != 0)[0]
    order = used[np.argsort(masks[used], kind="stable")]
    m_sorted = masks[order]
    i = 0
    while i + 1 < len(order):
        if m_sorted[i] == m_sorted[i + 1]:
            u, v = order[i], order[i + 1]
            partner[u] = v
            partner[v] = u
            i += 2
        else:
            i += 1
    return partner


def _core_half_layout(cfg, src, dst, core, half):
    """Build per-core pair/single descriptor lists per j for one half.

    Returns (perm, pair_descs[j], pair_uses[j], single_uses[j]) where
    pair_descs[j] = list of pair ids, pair_uses[j] = list of
    (dstoff_even, dstoff_odd) aligned with pair_descs, single_uses[j] =
    list of (row, dstoff).
    """
    NT2 = cfg.NT2
    masks, peruse = _half_masks_counts(cfg, src, dst, core, half)
    partner = _match_pairs(masks)

    # rows: paired nodes first (2p, 2p+1), then the rest
    rowof = np.full(NT2, -1, dtype=np.int64)
    perm = np.zeros(NT2, dtype=np.int64)
    r = 0
    for u in range(NT2):
        v = partner[u]
        if v > u:
            perm[r] = u
            perm[r + 1] = v
            rowof[u] = r
            rowof[v] = r + 1
            r += 2
    npair_rows = r
    for u in range(NT2):
        if rowof[u] < 0:
            perm[r] = u
            rowof[u] = r
            r += 1

    pair_descs = []
    pair_uses = []
    single_uses = []
    for j in range(cfg.NSB):
        es, ed = peruse[j]
        pd = []
        pu = []
        su = []
        # group uses by node
        o = np.argsort(es, kind="stable")
        es_s, ed_s = es[o], ed[o]
        starts = {}
        idx = 0
        while idx < len(es_s):
            n = es_s[idx]
            k = idx
            while k < len(es_s) and es_s[k] == n:
                k += 1
            starts[n] = (idx, k)
            idx = k
        done = set()
        for n, (a, b) in starts.items():
            if n in done:
                continue
            v = partner[n]
            if v >= 0 and v in starts:
                u_, v_ = (n, v) if rowof[n] < rowof[v] else (v, n)
                (ua, ub) = starts[u_]
                (va, vb) = starts[v_]
                c = min(ub - ua, vb - va)
                pid = rowof[u_] // 2
                for t in range(c):
                    pd.append(pid)
                    pu.append((ed_s[ua + t], ed_s[va + t]))
                for t in range(c, ub - ua):
                    su.append((rowof[u_], ed_s[ua + t]))
                for t in range(c, vb - va):
                    su.append((rowof[v_], ed_s[va + t]))
                done.add(u_)
                done.add(v_)
            else:
                for t in range(a, b):
                    su.append((rowof[n], ed_s[t]))
                done.add(n)
        # sort by table row for DRAM locality
        po = np.argsort(np.array(pd, dtype=np.int64), kind="stable") \
            if pd else np.array([], dtype=np.int64)
        pd = [pd[x] for x in po]
        pu = [pu[x] for x in po]
        su.sort()
        pair_descs.append(pd)
        pair_uses.append(pu)
        single_uses.append(su)
    return perm, pair_descs, pair_uses, single_uses


def make_cfg(N, IN, H, D, src, dst, n_cores=8):
    cfg = Cfg(N=N, IN=IN, H=H, D=D, n_cores=n_cores)
    cfg.NPC = -(-N // (n_cores * 128)) * 128
    cfg.NT = cfg.NPC * n_cores
    cfg.NSB = cfg.NPC // 128
    src = np.asarray(src)
    dst = np.asarray(dst)
    gsb = dst // 128
    nsb_tot = cfg.NSB * n_cores
    counts = np.zeros(nsb_tot, dtype=np.int64)
    np.add.at(counts, np.minimum(gsb, nsb_tot - 1), 1)
    order = np.argsort(-counts, kind="stable")
    cfg.ASSIGN = [[int(order[k * n_cores + i]) for k in range(cfg.NSB)]
                  for i in range(n_cores)]

    # per-core layouts to size the static regions
    cfg._core_half = {}
    for i in range(n_cores):
        for h in (0, 1):
            cfg._core_half[(i, h)] = _core_half_layout(cfg, src, dst, i, h)

    # static region layout: (half, cls) x j-subregions, desc counts =
    # max over cores, padded to multiples of 128 descriptors
    regions = []
    blk = 0
    desc = 0
    for h in (0, 1):
        for cls in ("p", "s"):
            jsub = []
            for j in range(cfg.NSB):
                mx = 0
                for i in range(n_cores):
                    _, pdj, _, suj = cfg._core_half[(i, h)]
                    n = len(pdj[j]) if cls == "p" else len(suj[j])
                    mx = max(mx, n)
                nd = max(128, -(-mx // 128) * 128)
                jsub.append((j, nd))
            tot_desc = sum(nd for _, nd in jsub)
            spb = 64 if cls == "p" else 128       # descs per block
            tot_blk = tot_desc // spb
            # chunks: cut at <=CH_CAP blocks on j boundaries crossing ok,
            # but block counts per j are even for pairs so any even cut works
            chunks = []
            # build block->j map for the region
            bj = []
            for j, nd in jsub:
                bj += [j] * (nd // spb)
            b0 = 0
            while b0 < tot_blk:
                nb = min(CH_CAP, tot_blk - b0)
                if cls == "p" and nb % 2 == 1:
                    nb -= 1
                segs = []
                for b in range(b0, b0 + nb):
                    if segs and segs[-1][0] == bj[b]:
                        segs[-1][2] += 1
                    else:
                        segs.append([bj[b], b - b0, 1])
                chunks.append({
                    "blk0": blk + b0,
                    "nblk": nb,
                    "desc0": desc + b0 * spb,
                    "ndesc": nb * spb,
                    "segs": [tuple(s) for s in segs],
                })
                b0 += nb
            regions.append({"h": h, "cls": cls, "jsub": jsub,
                            "chunks": chunks, "spb": spb})
            blk += tot_blk
            desc += tot_desc
    cfg.REGIONS = regions
    cfg.SUMBLK = blk
    cfg.SUMDESC = desc
    return cfg


def _wrap16(idx, n):
    base = idx.reshape(n // 16, 16).T.astype(np.int16)
    return np.tile(base, (8, 1))


def prep(cfg: Cfg, h, Wq, bq, Wk, bk, Wv, bv, src, dst):
    N, IN, H, D, C = cfg.N, cfg.IN, cfg.H, cfg.D, cfg.C
    NT2 = cfg.NT2
    scale = 1.0 / np.sqrt(np.float32(D))

    hT = np.zeros((IN, cfg.NT), dtype=BF16)
    hT[:, :N] = np.asarray(h).T.astype(BF16)
    wkv = np.concatenate([np.asarray(Wk), np.asarray(Wv)], axis=1).astype(BF16)
    wq = (np.asarray(Wq) * scale).astype(BF16)

    bq_rep = np.tile((np.asarray(bq) * scale)[None, :], (128, 1)).astype(BF16)
    bk_rep = np.tile(np.asarray(bk)[None, :], (128, 1)).astype(BF16)
    bv_rep = np.tile(np.asarray(bv)[None, :], (128, 1)).astype(F32)

    src = np.asarray(src).astype(np.int64)
    dst = np.asarray(dst).astype(np.int64)

    marange = np.arange(128, dtype=np.int64)
    in_maps = []
    for i in range(cfg.n_cores):
        descidx = np.zeros(cfg.SUMDESC, dtype=np.int64)
        ld = np.full((cfg.SUMBLK, 128), 255, dtype=np.int64)

        perm_cols = np.zeros(cfg.NT, dtype=np.int64)
        for reg in cfg.REGIONS:
            hf = reg["h"]
            perm, pds, pus, sus = cfg._core_half[(i, hf)]
            perm_cols[hf * NT2:(hf + 1) * NT2] = perm + hf * NT2
            for (j, nd) in reg["jsub"]:
                pass  # offsets handled below

        # fill desc/ld arrays region by region
        for reg in cfg.REGIONS:
            hf = reg["h"]
            cls = reg["cls"]
            spb = reg["spb"]
            perm, pds, pus, sus = cfg._core_half[(i, hf)]
            # compute this region's desc/blk base from chunks[0]
            desc0 = reg["chunks"][0]["desc0"]
            blk0 = reg["chunks"][0]["blk0"]
            off_d = desc0
            off_b = blk0
            for (j, nd) in reg["jsub"]:
                if cls == "p":
                    pd = pds[j]
                    pu = pus[j]
                    cnt = len(pd)
                    descidx[off_d:off_d + cnt] = pd
                    # slot mapping: desc k -> partition k%128,
                    # pair-group k//128 -> blocks off_b + 2*(k//128) + {0,1}
                    for k in range(cnt):
                        p = k % 128
                        bg = k // 128
                        ld[off_b + 2 * bg, p] = pu[k][0]
                        ld[off_b + 2 * bg + 1, p] = pu[k][1]
                else:
                    su = sus[j]
                    cnt = len(su)
                    for k in range(cnt):
                        row, doff = su[k]
                        descidx[off_d + k] = row
                        ld[off_b + k // 128, k % 128] = doff
                off_d += nd
                off_b += nd // spb

        # wrap indices per chunk slice (wrap is global: 16-partition wrap
        # over each 16-desc run; chunks are 128-desc aligned so a single
        # global wrap works)
        srcw = _wrap16(descidx, cfg.SUMDESC)

        onehot = (ld[:, :, None] == marange[None, None, :])
        Sh = np.ascontiguousarray(onehot.transpose(1, 0, 2)).astype(FP8)
        ShT = np.ascontiguousarray(onehot.transpose(2, 0, 1)).astype(FP8)

        cols = np.concatenate(
            [np.arange(cfg.ASSIGN[i][j] * 128, cfg.ASSIGN[i][j] * 128 + 128)
             for j in range(cfg.NSB)])
        in_maps.append({
            "hT": np.ascontiguousarray(hT[:, perm_cols]),
            "hTq": np.ascontiguousarray(hT[:, cols]),
            "wkv": wkv, "wq": wq,
            "bq_rep": bq_rep, "bk_rep": bk_rep, "bv_rep": bv_rep,
            "srcidx": srcw,
            "Sh": Sh, "ShT": ShT,
        })
    return in_maps


def build(cfg: Cfg):
    N, IN, H, D, C = cfg.N, cfg.IN, cfg.H, cfg.D, cfg.C
    KA = cfg.KA
    C2 = 2 * C
    CZ = cfg.CZ
    bf = mybir.dt.bfloat16
    f8 = mybir.dt.float8e4
    f32 = mybir.dt.float32

    nc = bacc.Bacc("TRN2", target_bir_lowering=False, debug=False)
    hT = nc.dram_tensor("hT", [IN, cfg.NT], bf, kind="ExternalInput").ap()
    hTq = nc.dram_tensor("hTq", [IN, cfg.NPC], bf, kind="ExternalInput").ap()
    wkv = nc.dram_tensor("wkv", [IN, C2], bf, kind="ExternalInput").ap()
    wq = nc.dram_tensor("wq", [IN, C], bf, kind="ExternalInput").ap()
    bq_rep = nc.dram_tensor("bq_rep", [128, C], bf, kind="ExternalInput").ap()
    bk_rep = nc.dram_tensor("bk_rep", [128, C], bf, kind="ExternalInput").ap()
    bv_rep = nc.dram_tensor("bv_rep", [128, C], f32, kind="ExternalInput").ap()
    srcidx = nc.dram_tensor("srcidx", [128, cfg.SUMDESC // 16], mybir.dt.int16,
                            kind="ExternalInput").ap()
    Sh_d = nc.dram_tensor("Sh", [128, cfg.SUMBLK, 128], f8,
                          kind="ExternalInput").ap()
    ShT_d = nc.dram_tensor("ShT", [128, cfg.SUMBLK, 128], f8,
                           kind="ExternalInput").ap()
    out = nc.dram_tensor("out", [cfg.NPC, C], f32, kind="ExternalOutput").ap()

    with tile.TileContext(nc) as tc:
        with (
            tc.tile_pool(name="dram", bufs=1, space="DRAM") as dramp,
            tc.tile_pool(name="const", bufs=1) as constp,
        ):
            kv_lo = dramp.tile([cfg.NT2, C2], bf)
            kv_hi = dramp.tile([cfg.NT2, C2], bf)

            wkvt = constp.tile([128, KA, C2], bf)
            nc.sync.dma_start(wkvt[:], wkv.rearrange("(a p) c -> p a c", p=128))
            wqt = constp.tile([128, KA, C], bf)
            nc.sync.dma_start(wqt[:], wq.rearrange("(a p) c -> p a c", p=128))
            bqr = constp.tile([128, C], bf)
            nc.sync.dma_start(bqr[:], bq_rep[:])
            bkr = constp.tile([128, C], bf)
            nc.sync.dma_start(bkr[:], bk_rep[:])
            bvr = constp.tile([128, C], f32)
            nc.sync.dma_start(bvr[:], bv_rep[:])
            srct = constp.tile([128, cfg.SUMDESC // 16], mybir.dt.int16)
            nc.sync.dma_start(srct[:], srcidx[:])
            qs = constp.tile([128, cfg.NSB, CZ], bf)
            wvacc = constp.tile([128, cfg.NSB, CZ], f32)
            nc.vector.memset(wvacc[:], 0.0)

            # ---------------- Phase A ----------------
            import contextlib
            pg_ctx = contextlib.ExitStack()
            pg = pg_ctx.enter_context(tc.tile_pool(name="pb_g", bufs=4))
            with (
                tc.tile_pool(name="pa_h", bufs=1) as pah,
                tc.tile_pool(name="pa_ps", bufs=6, space="PSUM") as paps,
                tc.tile_pool(name="pa_sb", bufs=4) as pasb,
                tc.tile_pool(name="pa_t", bufs=2) as patp,
            ):
                NSPL = 8
                SPL = cfg.NT // NSPL
                hT_r = hT.rearrange("(a p) n -> p a n", p=128)
                hts = []
                for sp in range(NSPL):
                    t = pah.tile([128, KA, SPL], bf, name=f"hts{sp}")
                    nc.sync.dma_start(t[:], hT_r[:, :, ts(sp, SPL)])
                    hts.append(t)
                htq = pah.tile([128, KA, cfg.NPC], bf)
                nc.sync.dma_start(htq[:], hTq.rearrange("(a p) n -> p a n", p=128))

                NC2 = cfg.NT2 // 128
                CPS = SPL // 128   # node-chunks per split tile

                def kv_chunk(cc, tbl):
                    sp, rc = cc // CPS, cc % CPS
                    ps = paps.tile([128, C2], f32, tag="psA")
                    for a in range(KA):
                        nc.tensor.matmul(out=ps[:],
                                         lhsT=hts[sp][:, a, ts(rc, 128)],
                                         rhs=wkvt[:, a, :], start=(a == 0),
                                         stop=(a == KA - 1))
                    buf = pasb.tile([128, C2], bf, tag="bufA")
                    nc.scalar.copy(buf[:, 0:C], ps[:, 0:C])
                    nc.vector.tensor_copy(buf[:, C:C2], ps[:, C:C2])
                    nc.sync.dma_start(tbl[ts(cc % NC2, 128), :], buf[:])

                for cc in range(NC2):
                    kv_chunk(cc, kv_lo[:])

                for qc in range(cfg.NSB):
                    psq = paps.tile([128, C], f32, tag="psA", name="psq")
                    for a in range(KA):
                        nc.tensor.matmul(out=psq[:], lhsT=htq[:, a, ts(qc, 128)],
                                         rhs=wqt[:, a, :], start=(a == 0),
                                         stop=(a == KA - 1))
                    nc.vector.tensor_tensor(qs[:, qc, 0:C], psq[:], bqr[:],
                                            op=ALU.add)
                    tq = patp.tile([128, C], bf, tag="tq")
                    nc.vector.tensor_tensor(tq[:], qs[:, qc, 0:C], bkr[:],
                                            op=ALU.mult)
                    t4 = tq[:].rearrange("p (h d) -> p h d", d=D)
                    cur = t4
                    w = D
                    while w > 2:
                        w //= 2
                        nxt = patp.tile([128, H, w], bf, tag=f"tt{w}")
                        nc.vector.tensor_tensor(
                            nxt[:], cur[:, :, 0:w], cur[:, :, w:2 * w],
                            op=ALU.add)
                        cur = nxt[:]
                    nc.vector.tensor_tensor(
                        qs[:, qc, C:CZ].unsqueeze(2), cur[:, :, 0:1],
                        cur[:, :, 1:2], op=ALU.add)

                for cc in range(NC2, 2 * NC2):
                    kv_chunk(cc, kv_hi[:])

            # ---------------- Phase B ----------------
            with (
                tc.tile_pool(name="pb_t", bufs=2) as pt,
                tc.tile_pool(name="pb_c", bufs=2) as pc,
                tc.tile_pool(name="pb_w", bufs=2) as pw,
                tc.tile_pool(name="pb_s", bufs=2) as psm,
                tc.tile_pool(name="pb_ps", bufs=4, space="PSUM") as pps,
                tc.tile_pool(name="pb_qps", bufs=4, space="PSUM") as pqps,
            ):
                def do_chunk(reg, ch):
                    cls = reg["cls"]
                    tbl = kv_lo[:] if reg["h"] == 0 else kv_hi[:]
                    CH = ch["nblk"]
                    nd = ch["ndesc"]
                    d0 = ch["desc0"]
                    kvg = pg.tile([128, CH_CAP, C2], bf, tag="kvg")
                    if cls == "p":
                        nc.gpsimd.dma_gather(
                            kvg[:, 0:CH, :].rearrange(
                                "p (a b) c -> p a (b c)", b=2),
                            tbl.rearrange("(a b) c -> a (b c)", b=2),
                            srct[:, d0 // 16:(d0 + nd) // 16],
                            nd, nd, 2 * C2, single_packet=False)
                    else:
                        nc.gpsimd.dma_gather(
                            kvg[:, 0:CH, :], tbl,
                            srct[:, d0 // 16:(d0 + nd) // 16],
                            nd, nd, C2, single_packet=False)
                    b0 = ch["blk0"]
                    sh = pt.tile([128, CH, 128], f8, tag="sh")
                    nc.sync.dma_start(sh[:], Sh_d[:, b0:b0 + CH, :])
                    sht = pt.tile([128, CH, 128], f8, tag="sht")
                    nc.sync.dma_start(sht[:], ShT_d[:, b0:b0 + CH, :])

                    qg = pc.tile([128, CH, CZ], bf, tag="qg")
                    for (j, br, nb) in ch["segs"]:
                        for b in range(br, br + nb):
                            qps = pqps.tile([128, 512], f32, tag="qps")
                            nc.tensor.matmul(out=qps[:, 0:CZ],
                                             lhsT=sht[:, b, :],
                                             rhs=qs[:, j, :],
                                             start=True, stop=True)
                            nc.scalar.copy(qg[:, b, :], qps[:, 0:CZ])

                    P = pc.tile([128, CH, C], bf, tag="P")
                    nc.vector.tensor_tensor(P[:, 0:CH, :], kvg[:, 0:CH, 0:C],
                                            qg[:, :, 0:C], op=ALU.mult)
                    P4 = P[:, 0:CH, :].rearrange("p b (h d) -> p b h d", d=D)
                    cur = P4
                    w = D
                    while w > 2:
                        w //= 2
                        nxt = pc.tile([128, CH, H, w], bf, tag=f"s{w}")
                        nc.vector.tensor_tensor(
                            nxt[:, 0:CH], cur[:, :, :, 0:w],
                            cur[:, :, :, w:2 * w], op=ALU.add)
                        cur = nxt[:, 0:CH]
                    sc = pc.tile([128, CH, H], f32, tag="sc")
                    nc.vector.tensor_tensor(
                        sc[:, 0:CH].unsqueeze(3), cur[:, :, :, 0:1],
                        cur[:, :, :, 1:2], op=ALU.add)
                    sc2 = pc.tile([128, CH, H], f32, tag="sc2")
                    nc.vector.tensor_tensor(sc2[:, 0:CH], sc[:, 0:CH],
                                            qg[:, :, C:CZ], op=ALU.add)

                    wvz = pw.tile([128, CH, CZ], bf, tag="wvz")
                    nc.scalar.activation(wvz[:, 0:CH, C:CZ], sc2[:, 0:CH],
                                         AF.Exp)
                    nc.vector.tensor_tensor(
                        wvz[:, 0:CH, 0:C].rearrange("p b (h d) -> p b h d", d=D),
                        kvg[:, 0:CH, C:C2].rearrange("p b (h d) -> p b h d", d=D),
                        wvz[:, 0:CH, C:CZ].unsqueeze(3).broadcast_to(
                            [128, CH, H, D]),
                        op=ALU.mult)

                    for (j, br, nb) in ch["segs"]:
                        pz = pps.tile([128, 512], f32, tag="pz")
                        for b in range(br, br + nb):
                            nc.tensor.matmul(
                                out=pz[:, 0:CZ], lhsT=sh[:, b, :],
                                rhs=wvz[:, b, :],
                                start=(b == br), stop=(b == br + nb - 1))
                        nc.vector.tensor_tensor(wvacc[:, j, :], wvacc[:, j, :],
                                                pz[:, 0:CZ], op=ALU.add)

                for reg in cfg.REGIONS:
                    for ch in reg["chunks"]:
                        do_chunk(reg, ch)

                def finalize(j):
                    zm = psm.tile([128, H], f32, tag="zm")
                    nc.vector.tensor_scalar(zm[:], wvacc[:, j, C:CZ], 1e-30,
                                            None, op0=ALU.max)
                    zr = psm.tile([128, H], f32, tag="zr")
                    nc.vector.reciprocal(zr[:], zm[:])
                    zbv = psm.tile([128, C], f32, tag="zbv")
                    nc.vector.tensor_tensor(
                        zbv[:].rearrange("p (h d) -> p h d", d=D),
                        bvr[:].rearrange("p (h d) -> p h d", d=D),
                        wvacc[:, j, C:CZ].unsqueeze(2).broadcast_to([128, H, D]),
                        op=ALU.mult)
                    wv2 = psm.tile([128, C], f32, tag="wv2")
                    nc.vector.tensor_tensor(wv2[:], wvacc[:, j, 0:C], zbv[:],
                                            op=ALU.add)
                    of = psm.tile([128, C], f32, tag="of")
                    nc.vector.tensor_tensor(
                        of[:].rearrange("p (h d) -> p h d", d=D),
                        wv2[:].rearrange("p (h d) -> p h d", d=D),
                        zr[:].unsqueeze(2).broadcast_to([128, H, D]),
                        op=ALU.mult)
                    nc.sync.dma_start(out[ts(j, 128), :], of[:])

                for j in range(cfg.NSB):
                    finalize(j)
            pg_ctx.close()

    nc.compile()
    return nc


def run(cfg: Cfg, in_maps, trace=False, nc=None):
    if nc is None:
        nc = build(cfg)
    res = run_bass_kernel_spmd(nc, in_maps, core_ids=list(range(cfg.n_cores)),
                               trace=trace)
    full = np.zeros((cfg.NT, cfg.C), dtype=np.float32)
    for i in range(cfg.n_cores):
        o = res.results[i]["out"]
        for j in range(cfg.NSB):
            g_sb = cfg.ASSIGN[i][j]
            full[g_sb * 128:(g_sb + 1) * 128] = o[j * 128:(j + 1) * 128]
    full = full[:cfg.N]
    return full.reshape(cfg.N, cfg.H, cfg.D), res


_PROBLEM_N = 10000
_PROBLEM_IN = 256
_PROBLEM_H = 8
_PROBLEM_D = 32


def kernel(h, Wq, bq, Wk, bk, Wv, bv, src, dst):
    h = np.asarray(h)
    N, IN = h.shape
    C = np.asarray(Wq).shape[1]
    H, D = _PROBLEM_H, _PROBLEM_D
    if C != H * D:
        D = C // H
    src = np.asarray(src)
    dst = np.asarray(dst)
    cfg = make_cfg(N, IN, H, D, src, dst)
    in_maps = prep(cfg, h, Wq, bq, Wk, bk, Wv, bv, src, dst)
    out, _ = run(cfg, in_maps, trace=False)
    return out.astype(np.float32)
